# revision 23
# baseline (speedup 1.0000x reference)
"""Trainium2 Bass kernel for DirectionalSeparableConv2D.

Full-input contract: kernel(**inputs) takes the complete unsharded inputs
(x [128,128,48,48] plus the small weight tensors) and returns the full
[128,128,48,48] output. Internally shards batch 16-per-core across 8
NeuronCores (SPMD), with all weights replicated.

Math: the whole module is linear in x.
  stage 1 (depthwise): central group (ch 0:32) 3x3 kernels; four
    directional groups (24 ch each) 5-tap line kernels along
    horizontal / vertical / diagonal / anti-diagonal directions.
  stage 2: 1x1 mixing, representable as a single [128,128] matrix Mix.

Kernel strategy per core (v3 — all-bf16, engine-balanced):
  - channels on SBUF partitions, pixels on the free dim; 4-image blocks
    with image i in partition slot [32i, 32i+32).
  - PE: 20 packs per 8-row chunk — 9 central taps and 7 dir taps as
    (Mix*k)@shift(x) fused matmuls, plus 4 stage-2 packs mixing the dir
    depthwise partials y'. 4 images run concurrently via 32-row
    tile_position packing.
  - y' per group accumulates the remaining taps elementwise: the center
    tap t=2 is the chain head on ScalarE (full region -> no border
    memsets); taps t=0,t=4 (and t=3 of the vertical group) are
    scalar_tensor_tensor MACs on VectorE (STT runs at 1 elem/cyc on
    trn2 DVE regardless of dtype — no 2x uop exists for it).
  - everything is bf16 (x, y', weights, staging, output); PSUM
    accumulates in f32. Host casts the bf16 output back to f32.
  - evac: one batched PSUM->SBUF copy per chunk ([128, 4x384] strided
    out of a bank-padded [128, 4x512] PSUM tile), mostly on ScalarE;
    stores every 2 chunks, alternating HWDGE rings.
  - emission is staggered one block ahead (loads + y'-prep of block b+1
    before PE of block b) so the PE never head-of-line blocks.
"""

import numpy as np

import concourse.bacc as bacc
import concourse.mybir as mybir
import concourse.tile as tile
from concourse.bass_utils import run_bass_kernel_spmd

F32 = mybir.dt.float32
BF16 = mybir.dt.bfloat16

# Problem shapes (hardcoded per contract).
B, C, H, W = 128, 128, 48, 48
HW = H * W
CEN_IN, DIR_IN = 32, 24
N_CORES = 8

# Per-core tiling.
NB = B // N_CORES          # images per core (16)
IPB = 4                    # images per block (one per 32-partition slot)
NBLK = NB // IPB           # 4 blocks
RPC = 8                    # rows per PSUM chunk
CHUNK = RPC * W            # 384
PCH = 512                  # PSUM per-image slice stride (bank aligned)
NCH = H // RPC             # 6 chunks per image
SGRP = 2                   # chunks buffered per output store
PR = 96 + DIR_IN           # partition extent covering all dir slots (120)
HH = H // 2

# Dir-group tap geometry: group g shift for tap t (t=0..4, center t=2).
#   g=0 horizontal (0, t-2); g=1 vertical (t-2, 0);
#   g=2 diagonal (t-2, t-2); g=3 anti-diagonal (t-2, 2-t).
def dir_shift(g, t):
    d = t - 2
    return [(0, d), (d, 0), (d, d), (d, -d)][g]

# Split of the 16 non-center dir taps: PE-fused packs vs VectorE MACs.
PE_TAPS = [(0, 1), (0, 3), (1, 1), (2, 1), (2, 3), (3, 1), (3, 3)]
DVE_TAPS = [(0, 0), (1, 0), (2, 0), (3, 0),
            (0, 4), (1, 4), (2, 4), (3, 4), (1, 3)]
SCAL_TAPS = [0, 3, 4]      # scalar columns needed by the DVE MACs

# bf16 weight bundle layout.
WB_CEN = 0                 # 9 central taps x 128 cols
WB_DIR = 9 * 128           # len(PE_TAPS) fused dir taps x 128 cols
WB_S2 = WB_DIR + len(PE_TAPS) * 128   # 4 stage-2 blocks x 128 cols
WB_SCAL = WB_S2 + 4 * 128  # scalar columns for DVE taps
NWTB = WB_SCAL + len(SCAL_TAPS)


def build_mix(cen2cen, par2cen, dia2cen, cen2dir, dir2dir):
    mix = np.zeros((C, C), np.float32)
    mix[0:32, 0:32] = cen2cen
    mix[0:32, 32:56] = par2cen
    mix[0:32, 56:80] = par2cen
    mix[0:32, 80:104] = dia2cen
    mix[0:32, 104:128] = dia2cen
    for g in range(4):
        r = 32 + 24 * g
        mix[r:r + 24, 0:32] = cen2dir
        mix[r:r + 24, r:r + 24] = dir2dir
    return mix


def build_weights(cen_tensor, dir_tensor, cen2cen, par2cen, dia2cen, cen2dir, dir2dir):
    mix = build_mix(cen2cen, par2cen, dia2cen, cen2dir, dir2dir)
    bf = mybir.dt.np(BF16)
    wtb = np.zeros((128, NWTB), bf)
    for t in range(9):
        blk = (mix[:, 0:32] * cen_tensor[:, t // 3, t % 3][None, :]).T.astype(bf)
        for i in range(IPB):
            wtb[32 * i:32 * i + 32, WB_CEN + 128 * t:WB_CEN + 128 * (t + 1)] = blk
    for j, (g, t) in enumerate(PE_TAPS):
        cols = slice(32 + 24 * g, 56 + 24 * g)
        blk = (mix[:, cols] * dir_tensor[:, t][None, :]).T.astype(bf)
        for i in range(IPB):
            wtb[32 * i:32 * i + 24, WB_DIR + 128 * j:WB_DIR + 128 * (j + 1)] = blk
    for g in range(4):
        s2 = mix[:, 32 + 24 * g:56 + 24 * g].T.astype(bf)
        for i in range(IPB):
            wtb[32 * i:32 * i + 24, WB_S2 + 128 * g:WB_S2 + 128 * (g + 1)] = s2
    for j, t in enumerate(SCAL_TAPS):
        for i in range(IPB):
            wtb[32 * i:32 * i + 24, WB_SCAL + j] = dir_tensor[:, t].astype(bf)
    wts = np.zeros((128, 1), np.float32)
    for i in range(IPB):
        wts[32 * i:32 * i + 24, 0] = dir_tensor[:, 2]
    return wtb, wts


def build_nc(nb=NB):
    """Emit the per-core Bass program for nb images."""
    assert nb % IPB == 0
    nblk = nb // IPB
    nc = bacc.Bacc("TRN2", target_bir_lowering=False, debug=False)

    x = nc.dram_tensor("x", [nb * C * HW], BF16, kind="ExternalInput")
    wtbd = nc.dram_tensor("wtb", [128, NWTB], BF16, kind="ExternalInput")
    wtsd = nc.dram_tensor("wts", [128, 1], F32, kind="ExternalInput")
    out = nc.dram_tensor("out", [nb, C, HW], BF16, kind="ExternalOutput")

    xv = x[:].rearrange("(b c f) -> b c f", c=C, f=HW)
    xc = x[:].rearrange("(bc f) -> bc f", f=HW)

    MULT = mybir.AluOpType.mult
    ADD = mybir.AluOpType.add

    with tile.TileContext(nc) as tc:
        with (
            tc.tile_pool(name="wpool", bufs=1) as wpool,
            tc.tile_pool(name="cpool", bufs=4) as cpool,
            tc.tile_pool(name="dpool", bufs=4) as dpool,
            tc.tile_pool(name="ypool", bufs=3) as ypool,
            tc.tile_pool(name="spool", bufs=4) as spool,
            tc.tile_pool(name="ppool", bufs=2, space="PSUM") as ppool,
        ):
            wtb = wpool.tile([128, NWTB], BF16)
            nc.scalar.dma_start(out=wtb[:, :], in_=wtbd[:, :])
            wts = wpool.tile([128, 1], F32)
            nc.scalar.dma_start(out=wts[:, :], in_=wtsd[:, :])
            scal = {t: wtb[0:PR, WB_SCAL + j:WB_SCAL + j + 1]
                    for j, t in enumerate(SCAL_TAPS)}
            scal2f = wts[0:PR, 0:1]

            cen4_t, dir4_t, y4_t = {}, {}, {}

            def emit_loads(b):
                b0 = b * IPB
                cen4 = cpool.tile([128, HW], BF16, name=f"cen4_{b}", tag="cen4")
                dir4 = dpool.tile([128, 4 * HW], BF16, name=f"dir4_{b}", tag="dir4")
                cen4_t[b], dir4_t[b] = cen4, dir4
                nc.scalar.dma_start(out=cen4[:, :], in_=xv[b0:b0 + IPB, 0:32, :])
                for i in range(IPB):
                    bc = (b0 + i) * C
                    src = xc[bc + 32:bc + 128, :].rearrange("(g c) f -> c g f", g=4)
                    nc.sync.dma_start(out=dir4[32 * i:32 * i + 24, :], in_=src)

            def make_yprep(b, nsl=2):
                """Build the y'-prep op list (emission deferred for interleaving)."""
                dir4 = dir4_t[b]
                y4 = ypool.tile([128, 4 * HW], BF16, name=f"y4_{b}", tag="y4")
                y4_t[b] = y4
                d4v = dir4[:, :].rearrange("p (g h w) -> p g h w", g=4, w=W)
                y4v = y4[:, :].rearrange("p (g h w) -> p g h w", g=4, w=W)
                ops = []
                for sl in range(nsl):
                    h0, h1 = H * sl // nsl, H * (sl + 1) // nsl
                    # chain head: y = k2 * x (full region, ScalarE)
                    for g in range(4):
                        ops.append(lambda g=g, h0=h0, h1=h1: nc.scalar.mul(
                            y4[0:PR, g * HW + h0 * W:g * HW + h1 * W],
                            dir4[0:PR, g * HW + h0 * W:g * HW + h1 * W],
                            scal2f))
                    for g, t in DVE_TAPS:
                        dy, dx = dir_shift(g, t)
                        rl = max(max(0, -dy), h0)
                        rh = min(H - max(0, dy), h1)
                        cl, ch = max(0, -dx), W - max(0, dx)
                        if cl == 0 and ch == W:
                            src = dir4[0:PR,
                                       g * HW + (rl + dy) * W:g * HW + (rh + dy) * W]
                            dst = y4[0:PR, g * HW + rl * W:g * HW + rh * W]
                        else:
                            src = d4v[0:PR, g, rl + dy:rh + dy, cl + dx:ch + dx]
                            dst = y4v[0:PR, g, rl:rh, cl:ch]
                        ops.append(lambda src=src, dst=dst, t=t:
                                   nc.vector.scalar_tensor_tensor(
                                       out=dst, in0=src, scalar=scal[t], in1=dst,
                                       op0=MULT, op1=ADD))
                return ops

            def emit_pe(b, bg_ops=()):
                bg_ops = list(bg_ops)
                nbg = len(bg_ops)
                b0 = b * IPB
                cen4, dir4, y4 = cen4_t[b], dir4_t[b], y4_t[b]
                cen4v = cen4[:, :].rearrange("p (h w) -> p h w", w=W)
                d4v = dir4[:, :].rearrange("p (g h w) -> p g h w", g=4, w=W)
                stag = None
                for chk in range(NCH):
                    r0 = chk * RPC
                    c0 = r0 * W
                    pt = ppool.tile([128, IPB * PCH], F32,
                                    name=f"ps_{b}_{chk}", tag="ps")

                    def mm_tap(wcol, kk, rhs_flat, rhs_v, goff, dy, dx, first=False):
                        rl = max(r0, -dy)
                        rh = min(r0 + RPC, H - max(0, dy))
                        cl, ch = max(0, -dx), W - max(0, dx)
                        wsl = wtb[:, wcol:wcol + 128]
                        for i in range(IPB):
                            p0 = 32 * i
                            if cl == 0 and ch == W:
                                o = pt[:, i * PCH + rl * W - c0:
                                       i * PCH + rh * W - c0]
                                r = rhs_flat[p0:p0 + kk,
                                             goff + (rl + dy) * W:goff + (rh + dy) * W]
                            else:
                                o = pt[:, i * PCH:i * PCH + CHUNK].rearrange(
                                    "p (h w) -> p h w", w=W)[:, rl - r0:rh - r0, cl:ch]
                                r = rhs_v[p0:p0 + kk, rl + dy:rh + dy, cl + dx:ch + dx]
                            nc.tensor.matmul(
                                o, wsl[p0:p0 + kk, :], r,
                                start=first, stop=False, tile_position=(p0, 0))

                    mm_tap(WB_CEN + 128 * 4, 32, cen4, cen4v, 0, 0, 0, first=True)
                    for t in (0, 1, 2, 3, 5, 6, 7, 8):
                        mm_tap(WB_CEN + 128 * t, 32, cen4, cen4v,
                               0, t // 3 - 1, t % 3 - 1)
                    for j, (g, t) in enumerate(PE_TAPS):
                        dy, dx = dir_shift(g, t)
                        mm_tap(WB_DIR + 128 * j, 24, dir4, d4v[:, g], g * HW, dy, dx)
                    for g in range(4):
                        for i in range(IPB):
                            nc.tensor.matmul(
                                pt[:, i * PCH:i * PCH + CHUNK],
                                wtb[32 * i:32 * i + 24,
                                    WB_S2 + 128 * g:WB_S2 + 128 * (g + 1)],
                                y4[32 * i:32 * i + 24, g * HW + c0:g * HW + c0 + CHUNK],
                                start=False, stop=(g == 3), tile_position=(32 * i, 0))

                    # batched evacuation: one strided copy for all 4 images.
                    j = chk % SGRP
                    if j == 0:
                        stag = spool.tile([128, IPB * SGRP * CHUNK], BF16,
                                          name=f"st_{b}_{chk}", tag="st")
                    dstv = stag[:, :].rearrange(
                        "p (i f) -> p i f", i=IPB)[:, :, j * CHUNK:(j + 1) * CHUNK]
                    srcv = pt[:, :].rearrange("p (i f) -> p i f", i=IPB)[:, :, 0:CHUNK]
                    nc.scalar.copy(dstv, srcv)
                    if j == SGRP - 1:
                        lo = (chk - SGRP + 1) * CHUNK
                        dst = out[b0:b0 + IPB, :,
                                  lo:lo + SGRP * CHUNK].transpose([1, 0, 2])
                        st_src = stag[:, :].rearrange("p (i f) -> p i f", i=IPB)
                        nc.sync.dma_start(out=dst, in_=st_src)
                    # drain a slice of the next block's y'-prep ops so their
                    # static order interleaves with this block's evacs.
                    take = nbg * (chk + 1) // NCH - nbg * chk // NCH
                    for _ in range(take):
                        bg_ops.pop(0)()

            emit_loads(0)
            emit_loads(1)
            for op in make_yprep(0, nsl=4):
                op()
            for b in range(nblk):
                if b + 2 < nblk:
                    emit_loads(b + 2)
                bg = make_yprep(b + 1) if b + 1 < nblk else []
                emit_pe(b, bg)

    nc.compile()
    return nc


_NC_CACHE = {}


def _get_nc(nb):
    if nb not in _NC_CACHE:
        _NC_CACHE[nb] = build_nc(nb)
    return _NC_CACHE[nb]


def make_in_maps(x, wtb_np, wts_np, nb=NB, n_cores=N_CORES):
    bf = mybir.dt.np(BF16)
    x = np.ascontiguousarray(x, np.float32).reshape(B, C, HW).astype(bf)
    in_maps = []
    for k in range(n_cores):
        xs = x[k * nb:(k + 1) * nb].ravel()
        in_maps.append({"x": xs, "wtb": wtb_np, "wts": wts_np})
    return in_maps


def kernel(x, cen_tensor, dir_tensor, cen2cen, par2cen, dia2cen, cen2dir, dir2dir,
           _trace=False):
    wtb_np, wts_np = build_weights(
        np.asarray(cen_tensor, np.float32), np.asarray(dir_tensor, np.float32),
        np.asarray(cen2cen, np.float32), np.asarray(par2cen, np.float32),
        np.asarray(dia2cen, np.float32), np.asarray(cen2dir, np.float32),
        np.asarray(dir2dir, np.float32))
    nc = _get_nc(NB)
    in_maps = make_in_maps(np.asarray(x), wtb_np, wts_np)
    res = run_bass_kernel_spmd(nc, in_maps, list(range(N_CORES)), trace=_trace)
    outs = [np.asarray(res.results[k]["out"]).astype(np.float32).reshape(NB, C, H, W)
            for k in range(N_CORES)]
    full = np.concatenate(outs, axis=0)
    if _trace:
        return full, res
    return full
